# revision 11
# baseline (speedup 1.0000x reference)
"""Multi-head attention (B=2, S=2048, D=768, H=12) on 8 Trainium2 NeuronCores.

Sharding: core c handles batch b=c//4 and heads 3*(c%4) .. 3*(c%4)+2
(r = c%4 is the core's rank within its 4-core batch group).

Each core:
  1. Projects K for all 3 heads (feature-major, transposed) over the full
     sequence, then Q for s_q quarter 0; Q for quarters 1-3 are emitted as
     PE filler inside the attention stream.  V is projected sequence-major
     with an appended ones-column (softmax denominator), interleaved with
     the first attention groups.
  2. Per (head, s_k chunk): scores^T = K @ Q^T (contraction head_dim=64,
     heads paired into PE row-groups), exp on ScalarE, then
     ctx^T_aug = V_aug^T @ exp(scores^T) accumulated over s_k, yielding the
     unnormalized context and softmax denominator together.
  3. Normalizes ctx^T for its 192 features and keeps it in SBUF.
  4. Partial output projection per quarter: y_part^T[768, 512] =
     Wo[192 own feats, :]^T @ ctx^T (+ bo on the group-lead core only),
     written to DRAM.
  5. Per-quarter 4-rank ReduceScatter(add) within the batch group: core r
     receives out-feature rows 192r:192(r+1) of the summed y^T for that
     quarter -> out[nq*192:(nq+1)*192, :].
Host assembles y[b, nq*512:(nq+1)*512, 192r:192(r+1)] = out_c[nq block].T.

All matmul operands are float32r (TF32-like, full PE rate); accumulation fp32.
"""
import sys

if "/opt/trn_rl_repo" not in sys.path:
    sys.path.insert(0, "/opt/trn_rl_repo")

import numpy as np

B, S, D, H = 2, 2048, 768, 12
HD = 64
P = 128
N_CORES = 8
HPC = 3          # heads per core
NQ = 4           # s_q chunks of 512
SK = 16          # s_k chunks of 128
KD = 6           # D chunks of 128
W = 512          # working free-dim chunk
GPQ = SK + SK // 2   # attention groups per quarter (16 pair + 8 solo)

_CACHE = {}


def _install_profile_shim():
    """run_bass_kernel_spmd(trace=True) needs antenv.axon_hooks; provide it."""
    import contextlib
    import ctypes
    import types

    if "antenv.axon_hooks" in sys.modules:
        return
    try:
        lib = ctypes.CDLL("/opt/axon/libaxon_pjrt.so")
    except OSError:
        return
    if not hasattr(lib, "axon_start_nrt_profile"):
        return
    lib.axon_start_nrt_profile.argtypes = [
        ctypes.POINTER(ctypes.c_int64),
        ctypes.c_size_t,
    ]
    lib.axon_start_nrt_profile.restype = ctypes.c_int64
    lib.axon_stop_nrt_profile.argtypes = [ctypes.c_char_p]
    lib.axon_stop_nrt_profile.restype = ctypes.c_int64

    @contextlib.contextmanager
    def _hook(output_dir, device_ids):
        import jax

        jax.devices()
        if device_ids:
            ids = (ctypes.c_int64 * len(device_ids))(*device_ids)
            rc = lib.axon_start_nrt_profile(ids, len(device_ids))
        else:
            rc = lib.axon_start_nrt_profile(None, 0)
        if rc != 0:
            raise RuntimeError(f"axon_start_nrt_profile rc={rc}")
        try:
            yield
        finally:
            n = lib.axon_stop_nrt_profile(str(output_dir).encode())
            if n < 0:
                raise RuntimeError(f"axon_stop_nrt_profile rc={n}")

    mod = types.ModuleType("antenv.axon_hooks")
    mod.get_axon_ntff_profile_hook = lambda: _hook
    mod.set_axon_ntff_profile_hook = lambda h: None
    sys.modules["antenv.axon_hooks"] = mod


def _build():
    import concourse.bass as bass
    from concourse import bacc
    import concourse.tile as tile
    import concourse.mybir as mybir

    f32r = mybir.dt.float32r
    f32 = mybir.dt.float32
    AF = mybir.ActivationFunctionType
    ALU = mybir.AluOpType

    nc = bacc.Bacc("TRN2", target_bir_lowering=False, debug=False,
                   num_devices=N_CORES)

    xT = nc.dram_tensor("xT", [D, S], f32r, kind="ExternalInput")
    w_qk = nc.dram_tensor("w_qk", [D, 384], f32r, kind="ExternalInput")
    b_qk = nc.dram_tensor("b_qk", [384, 1], f32, kind="ExternalInput")
    w_v = nc.dram_tensor("w_v", [D, 256], f32r, kind="ExternalInput")
    b_v = nc.dram_tensor("b_v", [1, 256], f32, kind="ExternalInput")
    w_o = nc.dram_tensor("w_o", [192, D], f32r, kind="ExternalInput")
    b_o = nc.dram_tensor("b_o", [D, 1], f32, kind="ExternalInput")
    zin = nc.dram_tensor("zin", [P, P], f32r, kind="ExternalInput")
    out = nc.dram_tensor("out", [NQ * 192, W], f32, kind="ExternalOutput")

    rs_in = nc.dram_tensor("rs_in", [NQ, D, W], f32)
    rs_out = nc.dram_tensor("rs_out", [NQ * 192, W], f32)

    groups2 = [[c for c in range(N_CORES) if c // 4 == g] for g in range(2)]

    with tile.TileContext(nc) as tc:
        with tc.tile_pool(name="const", bufs=1) as const, \
             tc.tile_pool(name="qkp", bufs=1) as qkp, \
             tc.tile_pool(name="vp", bufs=1) as vp, \
             tc.tile_pool(name="work", bufs=4) as work, \
             tc.tile_pool(name="ctxp", bufs=2) as ctxp, \
             tc.tile_pool(name="expp", bufs=4) as expp, \
             tc.tile_pool(name="outp", bufs=3) as outp:

            # ---- constant loads -------------------------------------------
            zeros_t = const.tile([P, P], f32r, tag="zeros")
            nc.sync.dma_start(out=zeros_t, in_=zin[:, :])
            wqk = []
            xt = []
            for k in range(KD):
                t = const.tile([P, 384], f32r, tag=f"wqk{k}")
                nc.sync.dma_start(out=t, in_=w_qk[k * P:(k + 1) * P, :])
                wqk.append(t)
            wv = []
            for k in range(KD):
                t = const.tile([P, 256], f32r, tag=f"wv{k}")
                nc.scalar.dma_start(out=t, in_=w_v[k * P:(k + 1) * P, :])
                wv.append(t)
            woA = const.tile([P, D], f32r, tag="woA")
            nc.sync.dma_start(out=woA, in_=w_o[0:128, :])
            woB = const.tile([64, D], f32r, tag="woB")
            nc.sync.dma_start(out=woB, in_=w_o[128:192, :])
            bo = []
            for m in range(KD):
                t = const.tile([P, 1], f32, tag=f"bo{m}")
                nc.gpsimd.dma_start(out=t, in_=b_o[m * P:(m + 1) * P, :])
                bo.append(t)
            bqk = []
            for m in range(3):
                t = const.tile([P, 1], f32, tag=f"bqk{m}")
                nc.gpsimd.dma_start(out=t, in_=b_qk[m * P:(m + 1) * P, :])
                bqk.append(t)
            bv = const.tile([P, 256], f32, tag="bv")
            bv_bcast = bass.AP(tensor=b_v[:, :].tensor, offset=0,
                               ap=[[0, P], [1, 256]])
            nc.gpsimd.dma_start(out=bv, in_=bv_bcast)

            # x loads: n-chunk-major so the first QK blocks can start early;
            # spread across engine DMA rings
            for k in range(KD):
                t = const.tile([P, S], f32r, tag=f"xt{k}", name=f"xt{k}")
                xt.append(t)
            dma_engs = [nc.scalar, nc.gpsimd, nc.sync]
            di = 0
            for n in range(NQ):
                for k in range(KD):
                    dma_engs[di % 3].dma_start(
                        out=xt[k][:, n * W:(n + 1) * W],
                        in_=xT[k * P:(k + 1) * P, n * W:(n + 1) * W])
                    di += 1

            # ---- persistent SBUF tiles ------------------------------------
            # qkt[0]: K_h0|K_h1  qkt[1]: Q_h0|Q_h1  qkt[2]: K_h2|Q_h2
            qkt = [qkp.tile([P, S], f32r, tag=f"qkt{m}", name=f"qkt{m}")
                   for m in range(3)]
            q2c = qkp.tile([64, S], f32r, tag="q2c")
            vsb = [vp.tile([P, 256], f32r, tag=f"v{s}", name=f"v{s}")
                   for s in range(SK)]
            # normalized ctx^T per quarter: [h0;h1] rows 0:128 and h2 rows
            # 0:64, double-buffered across quarters
            ctn0 = [ctxp.tile([P, W], f32r, tag=f"ctn0_{j}", name=f"ctn0_{j}")
                    for j in range(2)]
            ctn1 = [ctxp.tile([64, W], f32r, tag=f"ctn1_{j}", name=f"ctn1_{j}")
                    for j in range(2)]

            # ---- attention machinery --------------------------------------
            pc_tiles = {}
            cnt = {}
            norm_done = {}

            def normalize(pc, nq, h):
                rec = work.tile([1, W], f32, tag="rec")
                nc.vector.reciprocal(rec[0:1, :], pc[64:65, :])
                rb = work.tile([64, W], f32, tag="rb")
                nc.gpsimd.partition_broadcast(rb, rec[:1, :])
                if h < 2:
                    dst = ctn0[nq % 2][h * HD:(h + 1) * HD, :]
                else:
                    dst = ctn1[nq % 2][:, :]
                nc.vector.tensor_tensor(out=dst, in0=pc[0:64, :], in1=rb,
                                        op=ALU.mult)
                norm_done.setdefault(nq, set()).add(h)

            groups = []
            for nq in range(NQ):
                for sk in range(SK):
                    groups.append({"nq": nq, "chunks": [(0, sk), (1, sk)]})
                for sk in range(0, SK, 2):
                    groups.append({"nq": nq, "chunks": [(2, sk), (2, sk + 1)]})

            def emit_mm_s(gi, grp):
                nq = grp["nq"]
                eps = psE.tile([P, 2 * W], f32, tag="ea" if gi % 2 == 0
                               else "eb", name=f"eps{gi}")
                for j, (h, sk) in enumerate(grp["chunks"]):
                    if h == 0:
                        lhsT = qkt[0][0:64, sk * P:(sk + 1) * P]
                        rhs = qkt[1][0:64, nq * W:(nq + 1) * W]
                        tp = (0, 0)
                    elif h == 1:
                        lhsT = qkt[0][64:128, sk * P:(sk + 1) * P]
                        rhs = qkt[1][64:128, nq * W:(nq + 1) * W]
                        tp = (64, 0)
                    else:
                        lhsT = qkt[2][0:64, sk * P:(sk + 1) * P]
                        rhs = q2c[:, nq * W:(nq + 1) * W]
                        tp = (0, 0)
                    nc.tensor.matmul(eps[:, j * W:(j + 1) * W], lhsT, rhs,
                                     start=True, stop=True, tile_position=tp)
                esb = expp.tile([P, 2 * W], f32r, tag="e", name=f"esb{gi}")
                nc.scalar.activation(esb, eps, AF.Exp)
                return esb

            def emit_mm_c(grp, esb):
                nq = grp["nq"]
                for j, (h, sk) in enumerate(grp["chunks"]):
                    key = (nq, h)
                    if key not in pc_tiles:
                        pc_tiles[key] = psC.tile([65, W], f32, tag="pc",
                                                 name=f"pc{nq}_{h}")
                        cnt[key] = 0
                    nc.tensor.matmul(
                        pc_tiles[key],
                        vsb[sk][:, h * 65:h * 65 + 65],
                        esb[:, j * W:(j + 1) * W],
                        start=(cnt[key] == 0), stop=(cnt[key] == SK - 1))
                    cnt[key] += 1
                    if cnt[key] == SK:
                        normalize(pc_tiles[key], nq, h)
                        del pc_tiles[key]

            # ---- emission -------------------------------------------------
            with tc.tile_pool(name="ps_proj", bufs=4, space="PSUM") as psP:

                def emit_qk_block(m, n, pool, tag, first=False):
                    ps = pool.tile([P, W], f32, tag=tag,
                                   name=f"psqk{m}_{n}")
                    if first:
                        # zero-contribution warmup: keeps the PE busy while
                        # x DMAs land so HAM ramps; the two regions cover
                        # [0:512] so has_written is clean for the real
                        # accumulation below
                        for d in range(24):
                            if d % 2 == 0:
                                nc.tensor.matmul(
                                    ps[:, 0:384], zeros_t, wqk[0][:, :],
                                    start=(d == 0), stop=False,
                                    skip_group_check=True)
                            else:
                                nc.tensor.matmul(
                                    ps[:, 384:512], zeros_t,
                                    wqk[1][:, 0:128],
                                    start=(d == 1), stop=False,
                                    skip_group_check=True)
                    for k in range(KD):
                        nc.tensor.matmul(
                            ps,
                            wqk[k][:, m * P:(m + 1) * P],
                            xt[k][:, n * W:(n + 1) * W],
                            start=(k == 0 and not first),
                            stop=(k == KD - 1),
                            skip_group_check=first)
                    nc.vector.tensor_scalar_add(
                        qkt[m][:, n * W:(n + 1) * W], ps, bqk[m])
                    if m == 2:
                        nc.sync.dma_start(
                            out=q2c[:, n * W:(n + 1) * W],
                            in_=qkt[2][64:128, n * W:(n + 1) * W])

                def emit_v_block(sk, pool, tag):
                    ps = pool.tile([P, W], f32, tag=tag, name=f"psv{sk}")
                    for k in range(KD):
                        nc.tensor.matmul(
                            ps[:, 0:256],
                            xt[k][:, sk * P:(sk + 1) * P],
                            wv[k],
                            start=(k == 0), stop=(k == KD - 1))
                    nc.vector.tensor_tensor(out=vsb[sk], in0=ps[:, 0:256],
                                            in1=bv, op=ALU.add)

                # pre-phase: K for all heads over full S, Q and V for
                # quarter 0; ordered so each x n-chunk is consumed by
                # several blocks while later chunks stream in
                emit_qk_block(0, 0, psP, "proj", first=True)
                emit_qk_block(2, 0, psP, "proj")
                emit_qk_block(1, 0, psP, "proj")
                for sk in range(4):
                    emit_v_block(sk, psP, "proj")
                for n in range(1, NQ):
                    emit_qk_block(0, n, psP, "proj")
                    emit_qk_block(2, n, psP, "proj")

            with tc.tile_pool(name="ps_e", bufs=1, space="PSUM") as psE, \
                 tc.tile_pool(name="ps_c", bufs=3, space="PSUM") as psC, \
                 tc.tile_pool(name="ps_x", bufs=1, space="PSUM") as psX:

                def emit_proj_m(nq, m):
                    pp = psX.tile([P, W], f32, tag="pp", name=f"pp{nq}_{m}")
                    nc.tensor.matmul(pp, woA[:, m * P:(m + 1) * P],
                                     ctn0[nq % 2], start=True, stop=False)
                    nc.tensor.matmul(pp, woB[:, m * P:(m + 1) * P],
                                     ctn1[nq % 2], start=False, stop=True)
                    yt = outp.tile([P, W], f32, tag="yt")
                    nc.vector.tensor_scalar_add(yt, pp, bo[m])
                    nc.gpsimd.dma_start(
                        out=rs_in[nq, m * P:(m + 1) * P, :], in_=yt)

                def emit_rs(nq):
                    nc.gpsimd.collective_compute(
                        "ReduceScatter", ALU.add,
                        ins=[rs_in[nq]],
                        outs=[rs_out[nq * 192:(nq + 1) * 192, :]],
                        replica_groups=groups2)
                    nc.scalar.dma_start(
                        out=out[nq * 192:(nq + 1) * 192, :],
                        in_=rs_out[nq * 192:(nq + 1) * 192, :])

                # filler schedule: group index -> list of closures
                fillers = {}

                def add_filler(gi, fn):
                    fillers.setdefault(gi, []).append(fn)

                # remaining V blocks early (1 per group; V(sk) must land
                # before ctx of pair group sk, emitted at gi = sk + HS)
                for i, sk in enumerate(range(4, SK)):
                    add_filler(1 + i, lambda sk=sk:
                               emit_v_block(sk, psX, "pp"))
                # Q projection for quarters 1-3, well before needed
                for n in range(1, NQ):
                    add_filler(n * GPQ - 8, lambda n=n:
                               emit_qk_block(1, n, psX, "pp"))
                # partial output projection for quarter nq spread into
                # quarter nq+1's groups; quarter 3 is handled at the end
                for nq in range(NQ - 1):
                    for m in range(KD):
                        add_filler((nq + 1) * GPQ + 6 + 2 * m,
                                   lambda nq=nq, m=m: emit_proj_m(nq, m))
                    add_filler((nq + 1) * GPQ + 6 + 2 * KD,
                               lambda nq=nq: emit_rs(nq))

                HS = 3
                pending = []
                for gi, grp in enumerate(groups):
                    esb = emit_mm_s(gi, grp)
                    pending.append((grp, esb))
                    if gi >= HS:
                        emit_mm_c(*pending.pop(0))
                    for fn in fillers.pop(gi, []):
                        fn()
                while pending:
                    emit_mm_c(*pending.pop(0))
                for m in range(KD):
                    emit_proj_m(3, m)
                emit_rs(3)

    nc.compile()
    return nc


def _get_nc():
    if "nc" not in _CACHE:
        _install_profile_shim()
        _CACHE["nc"] = _build()
    return _CACHE["nc"]


def _make_in_maps(x, Wq, bq, Wk, bk, Wv, bv, Wo, bo):
    scale = np.float32(1.0 / np.sqrt(HD))
    f = np.float32
    x, Wq, bq, Wk, bk, Wv, bv, Wo, bo = [
        np.asarray(a, dtype=f) for a in (x, Wq, bq, Wk, bk, Wv, bv, Wo, bo)]

    in_maps = []
    for c in range(N_CORES):
        b = c // 4
        r = c % 4
        hs = r * HPC
        hh = [hs, hs + 1, hs + 2]

        def wc(Wm, h):
            return Wm[:, h * HD:(h + 1) * HD]

        def bc(bm, h):
            return bm[h * HD:(h + 1) * HD]

        xTb = np.ascontiguousarray(x[b].T)
        w_qk = np.concatenate(
            [wc(Wk, hh[0]), wc(Wk, hh[1]),
             wc(Wq, hh[0]) * scale, wc(Wq, hh[1]) * scale,
             wc(Wk, hh[2]), wc(Wq, hh[2]) * scale], axis=1)
        b_qk = np.concatenate(
            [bc(bk, hh[0]), bc(bk, hh[1]),
             bc(bq, hh[0]) * scale, bc(bq, hh[1]) * scale,
             bc(bk, hh[2]), bc(bq, hh[2]) * scale])[:, None]
        w_v = np.zeros((D, 256), dtype=f)
        b_v = np.zeros((1, 256), dtype=f)
        for i, h in enumerate(hh):
            w_v[:, i * 65:i * 65 + HD] = wc(Wv, h)
            b_v[0, i * 65:i * 65 + HD] = bc(bv, h)
            b_v[0, i * 65 + HD] = 1.0
        # Wo rows for this core's 192 ctx features
        w_o = Wo[192 * r:192 * (r + 1), :]
        b_o_core = bo[:, None] if r == 0 else np.zeros((D, 1), dtype=f)
        in_maps.append({
            "xT": np.ascontiguousarray(xTb),
            "w_qk": np.ascontiguousarray(w_qk),
            "b_qk": np.ascontiguousarray(b_qk),
            "w_v": w_v,
            "b_v": b_v,
            "w_o": np.ascontiguousarray(w_o),
            "b_o": np.ascontiguousarray(b_o_core),
            "zin": np.zeros((P, P), dtype=f),
        })
    return in_maps


def _assemble(results):
    y = np.empty((B, S, D), dtype=np.float32)
    for c in range(N_CORES):
        b = c // 4
        r = c % 4
        o = results[c]["out"]
        for nq in range(NQ):
            y[b, nq * W:(nq + 1) * W, 192 * r:192 * (r + 1)] = \
                o[nq * 192:(nq + 1) * 192, :].T
    return y


def kernel(x, Wq, bq, Wk, bk, Wv, bv, Wo, bo, _trace=False):
    from concourse.bass_utils import run_bass_kernel_spmd

    nc = _get_nc()
    in_maps = _make_in_maps(x, Wq, bq, Wk, bk, Wv, bv, Wo, bo)
    res = run_bass_kernel_spmd(nc, in_maps, list(range(N_CORES)),
                               trace=_trace)
    _CACHE["last_results"] = res
    return _assemble([{"out": res.results[c]["out"]} for c in range(N_CORES)])


# revision 17
# speedup vs baseline: 1.2892x; 1.2892x over previous
"""Multi-head attention (B=2, S=2048, D=768, H=12) on 8 Trainium2 NeuronCores.

Sharding: core c handles batch b=c//4 and heads 3*(c%4) .. 3*(c%4)+2
(r = c%4).  Each core:
  1. Projects Q,K (feature-major, transposed) and V (sequence-major, with an
     appended ones-column for the softmax denominator) for its 3 heads.
  2. Computes scores^T = K @ Q^T per head (contraction over head_dim=64, heads
     paired into PE row-groups), exp on ScalarE (scores are O(1), no max
     subtraction needed), then ctx^T_aug = V_aug^T @ exp(scores^T) which yields
     both the unnormalized context and the softmax denominator in one pass.
  3. Normalizes (fast approximate reciprocal), writes local ctx^T [192, 2048]
     to DRAM; one 8-rank AllGather per s_q quarter -> ctx^T for all heads of
     both batches for that quarter [1536, 512] in cc_all.
  4. As soon as quarter q's AllGather lands, indirect-gathers its batch's
     full ctx^T [768, 512] slab to SBUF (pure DMA, overlapped with attention).
  5. After the attention stream: output projection of its 192 OUT-feature
     slice for every quarter, y^T[192, 512] = Wo[:, own cols]^T @ slab + bo;
     quarters 0-2 project while AllGather #3 completes.
Host assembles y[b, q*512:(q+1)*512, 192r:192(r+1)] = out_c[q block].T.

All matmul operands are float32r (TF32-like, full PE rate); accumulation fp32.
"""
import sys

if "/opt/trn_rl_repo" not in sys.path:
    sys.path.insert(0, "/opt/trn_rl_repo")

import ml_dtypes
import numpy as np

B, S, D, H = 2, 2048, 768, 12
HD = 64
P = 128
N_CORES = 8
HPC = 3          # heads per core
NQ = 4           # s_q chunks of 512
SK = 16          # s_k chunks of 128
KD = 6           # D chunks of 128
W = 512          # working free-dim chunk

_CACHE = {}


def _install_profile_shim():
    """run_bass_kernel_spmd(trace=True) needs antenv.axon_hooks; provide it."""
    import contextlib
    import ctypes
    import types

    if "antenv.axon_hooks" in sys.modules:
        return
    try:
        lib = ctypes.CDLL("/opt/axon/libaxon_pjrt.so")
    except OSError:
        return
    if not hasattr(lib, "axon_start_nrt_profile"):
        return
    lib.axon_start_nrt_profile.argtypes = [
        ctypes.POINTER(ctypes.c_int64),
        ctypes.c_size_t,
    ]
    lib.axon_start_nrt_profile.restype = ctypes.c_int64
    lib.axon_stop_nrt_profile.argtypes = [ctypes.c_char_p]
    lib.axon_stop_nrt_profile.restype = ctypes.c_int64

    @contextlib.contextmanager
    def _hook(output_dir, device_ids):
        import jax

        jax.devices()
        if device_ids:
            ids = (ctypes.c_int64 * len(device_ids))(*device_ids)
            rc = lib.axon_start_nrt_profile(ids, len(device_ids))
        else:
            rc = lib.axon_start_nrt_profile(None, 0)
        if rc != 0:
            raise RuntimeError(f"axon_start_nrt_profile rc={rc}")
        try:
            yield
        finally:
            n = lib.axon_stop_nrt_profile(str(output_dir).encode())
            if n < 0:
                raise RuntimeError(f"axon_stop_nrt_profile rc={n}")

    mod = types.ModuleType("antenv.axon_hooks")
    mod.get_axon_ntff_profile_hook = lambda: _hook
    mod.set_axon_ntff_profile_hook = lambda h: None
    sys.modules["antenv.axon_hooks"] = mod


def _build():
    import concourse.bass as bass
    from concourse import bacc
    import concourse.tile as tile
    import concourse.mybir as mybir

    f32r = mybir.dt.float32r
    f32 = mybir.dt.float32
    bf16 = mybir.dt.bfloat16
    u32 = mybir.dt.uint32
    AF = mybir.ActivationFunctionType
    ALU = mybir.AluOpType

    nc = bacc.Bacc("TRN2", target_bir_lowering=False, debug=False,
                   num_devices=N_CORES)

    xT = nc.dram_tensor("xT", [D, S], f32r, kind="ExternalInput")
    w_qk = nc.dram_tensor("w_qk", [D, 384], f32r, kind="ExternalInput")
    b_qk = nc.dram_tensor("b_qk", [384, 1], f32, kind="ExternalInput")
    w_v = nc.dram_tensor("w_v", [D, 256], f32r, kind="ExternalInput")
    b_v = nc.dram_tensor("b_v", [1, 256], f32, kind="ExternalInput")
    w_o = nc.dram_tensor("w_o", [D, 192], bf16, kind="ExternalInput")
    b_o = nc.dram_tensor("b_o", [192, 1], f32, kind="ExternalInput")
    gidx = nc.dram_tensor("gidx", [NQ * D, 1], u32, kind="ExternalInput")
    zin = nc.dram_tensor("zin", [P, P], f32r, kind="ExternalInput")
    out = nc.dram_tensor("out", [NQ * 192, W], f32, kind="ExternalOutput")

    cc_in = nc.dram_tensor("cc_in", [NQ, HPC * HD, W], bf16)
    cc_all = nc.dram_tensor("cc_all", [NQ * N_CORES * HPC * HD, W], bf16,
                            addr_space="Shared")

    with tile.TileContext(nc) as tc:
        with tc.tile_pool(name="const", bufs=1) as const, \
             tc.tile_pool(name="qkp", bufs=1) as qkp, \
             tc.tile_pool(name="vp", bufs=1) as vp, \
             tc.tile_pool(name="work", bufs=4) as work, \
             tc.tile_pool(name="expp", bufs=4) as expp, \
             tc.tile_pool(name="gat", bufs=1) as gat, \
             tc.tile_pool(name="outp", bufs=3) as outp:

            # ---- constant loads -------------------------------------------
            zeros_t = const.tile([P, P], f32r, tag="zeros")
            nc.sync.dma_start(out=zeros_t, in_=zin[:, :])
            wqk = []
            xt = []
            for k in range(KD):
                t = const.tile([P, 384], f32r, tag=f"wqk{k}")
                nc.sync.dma_start(out=t, in_=w_qk[k * P:(k + 1) * P, :])
                wqk.append(t)
            for k in range(KD):
                t = const.tile([P, S], f32r, tag=f"xt{k}", name=f"xt{k}")
                xt.append(t)
            for k in range(KD):
                nc.scalar.dma_start(out=xt[k][:, 0:1024],
                                    in_=xT[k * P:(k + 1) * P, 0:1024])
            for k in range(KD):
                nc.scalar.dma_start(out=xt[k][:, 1024:2048],
                                    in_=xT[k * P:(k + 1) * P, 1024:2048])
            wv = []
            for k in range(KD):
                t = const.tile([P, 256], f32r, tag=f"wv{k}")
                nc.sync.dma_start(out=t, in_=w_v[k * P:(k + 1) * P, :])
                wv.append(t)
            bqk = []
            for m in range(3):
                t = const.tile([P, 1], f32, tag=f"bqk{m}")
                nc.sync.dma_start(out=t, in_=b_qk[m * P:(m + 1) * P, :])
                bqk.append(t)
            bv = const.tile([P, 256], f32, tag="bv")
            bv_bcast = bass.AP(tensor=b_v[:, :].tensor, offset=0,
                               ap=[[0, P], [1, 256]])
            nc.gpsimd.dma_start(out=bv, in_=bv_bcast)
            wo = []
            for k in range(KD):
                t = const.tile([P, 192], bf16, tag=f"wo{k}")
                nc.sync.dma_start(out=t, in_=w_o[k * P:(k + 1) * P, :])
                wo.append(t)
            boA = const.tile([P, 1], f32, tag="boA")
            nc.sync.dma_start(out=boA, in_=b_o[0:128, :])
            boB = const.tile([64, 1], f32, tag="boB")
            nc.sync.dma_start(out=boB, in_=b_o[128:192, :])
            gix = []
            for q in range(NQ):
                row = []
                for k in range(KD):
                    t = const.tile([P, 1], u32, tag=f"gix{q}_{k}")
                    nc.sync.dma_start(
                        out=t, in_=gidx[q * D + k * P:q * D + (k + 1) * P, :])
                    row.append(t)
                gix.append(row)

            # per-quarter gathered ctx^T slabs [768, 512] (6 x [128, 512])
            slab = []
            for q in range(NQ):
                row = []
                for k in range(KD):
                    t = gat.tile([P, W], bf16, tag=f"slab{q}_{k}",
                                 name=f"slab{q}_{k}")
                    row.append(t)
                slab.append(row)

            # ---- attention -----------------------------------------------
            # Chunk = one [s_k 128, s_q 512] score block for one head.
            # Groups of 2 chunks share a 2-bank PSUM tile so one ACT exp
            # covers 1024 columns (amortizes the ~352-cycle ACT overhead).
            # Software-pipelined emission: mm_s(g+1) is emitted before
            # mm_c(g) so the PE never stalls behind the ACT.
            qkt = [qkp.tile([P, S], f32r, tag=f"qkt{m}", name=f"qkt{m}")
                   for m in range(3)]
            q2c = qkp.tile([64, S], f32r, tag="q2c")
            vsb = [vp.tile([P, 256], f32r, tag=f"v{s}", name=f"v{s}")
                   for s in range(SK)]

            def normalize(pc, nq, h):
                rec = work.tile([1, W], f32, tag="rec")
                nc.vector.reciprocal_approx_fast(out=rec[0:1, :],
                                                 in_=pc[64:65, :])
                rb = work.tile([64, W], f32, tag="rb")
                nc.gpsimd.partition_broadcast(rb, rec[:1, :])
                ctx = work.tile([64, W], bf16, tag="ctx")
                nc.vector.tensor_tensor(out=ctx, in0=pc[0:64, :], in1=rb,
                                        op=ALU.mult)
                nc.gpsimd.dma_start(
                    out=cc_in[nq, h * HD:(h + 1) * HD, :],
                    in_=ctx)
                norm_done.setdefault(nq, set()).add(h)
                if norm_done[nq] == {0, 1, 2}:
                    nc.gpsimd.collective_compute(
                        "AllGather", ALU.bypass,
                        ins=[cc_in[nq]],
                        outs=[cc_all[nq * 1536:(nq + 1) * 1536, :]],
                        replica_groups=[list(range(N_CORES))])
                    # gather this quarter's full-batch ctx^T slab as soon
                    # as the AllGather lands (pure DMA, no PE involvement)
                    for k in range(KD):
                        nc.gpsimd.indirect_dma_start(
                            out=slab[nq][k],
                            out_offset=None,
                            in_=cc_all[:, :],
                            in_offset=bass.IndirectOffsetOnAxis(
                                ap=gix[nq][k][:, :1], axis=0),
                        )

            # build group list: per nq, pair phase then solo phase
            groups = []
            for nq in range(NQ):
                for sk in range(SK):
                    groups.append({"nq": nq, "chunks": [(0, sk), (1, sk)]})
                for sk in range(0, SK, 2):
                    groups.append({"nq": nq, "chunks": [(2, sk), (2, sk + 1)]})

            pc_tiles = {}
            cnt = {}
            norm_done = {}

            def emit_mm_s(gi, grp):
                nq = grp["nq"]
                eps = psE.tile([P, 2 * W], f32, tag="ea" if gi % 2 == 0
                               else "eb", name=f"eps{gi}")
                for j, (h, sk) in enumerate(grp["chunks"]):
                    if h == 0:
                        lhsT = qkt[0][0:64, sk * P:(sk + 1) * P]
                        rhs = qkt[1][0:64, nq * W:(nq + 1) * W]
                        tp = (0, 0)
                    elif h == 1:
                        lhsT = qkt[0][64:128, sk * P:(sk + 1) * P]
                        rhs = qkt[1][64:128, nq * W:(nq + 1) * W]
                        tp = (64, 0)
                    else:
                        lhsT = qkt[2][0:64, sk * P:(sk + 1) * P]
                        rhs = q2c[:, nq * W:(nq + 1) * W]
                        tp = (0, 0)
                    nc.tensor.matmul(eps[:, j * W:(j + 1) * W], lhsT, rhs,
                                     start=True, stop=True, tile_position=tp)
                esb = expp.tile([P, 2 * W], f32r, tag="e", name=f"esb{gi}")
                nc.scalar.activation(esb, eps, AF.Exp)
                return esb

            def emit_mm_c(grp, esb):
                nq = grp["nq"]
                for j, (h, sk) in enumerate(grp["chunks"]):
                    key = (nq, h)
                    if key not in pc_tiles:
                        pc_tiles[key] = psC.tile([65, W], f32, tag="pc",
                                                 name=f"pc{nq}_{h}")
                        cnt[key] = 0
                    nc.tensor.matmul(
                        pc_tiles[key],
                        vsb[sk][:, h * 65:h * 65 + 65],
                        esb[:, j * W:(j + 1) * W],
                        start=(cnt[key] == 0), stop=(cnt[key] == SK - 1))
                    cnt[key] += 1
                    if cnt[key] == SK:
                        normalize(pc_tiles[key], nq, h)

            with tc.tile_pool(name="ps_proj", bufs=4, space="PSUM") as psP:

                def emit_qk_block(n):
                    for m in range(3):
                        ps = psP.tile([P, W], f32, tag="proj",
                                      name=f"psqk{n}_{m}")
                        first = n == 0 and m == 0
                        if first:
                            # zero-contribution warmup: keeps the PE busy
                            # while x DMAs land so HAM reaches 2.4GHz; the
                            # two regions cover [0:512] so has_written is
                            # clean for the real accumulation below
                            for d in range(24):
                                if d % 2 == 0:
                                    nc.tensor.matmul(
                                        ps[:, 0:384], zeros_t, wqk[0][:, :],
                                        start=(d == 0), stop=False,
                                        skip_group_check=True)
                                else:
                                    nc.tensor.matmul(
                                        ps[:, 384:512], zeros_t,
                                        wqk[1][:, 0:128],
                                        start=(d == 1), stop=False,
                                        skip_group_check=True)
                        for k in range(KD):
                            nc.tensor.matmul(
                                ps,
                                wqk[k][:, m * P:(m + 1) * P],
                                xt[k][:, n * W:(n + 1) * W],
                                start=(k == 0 and not first),
                                stop=(k == KD - 1),
                                skip_group_check=first)
                        nc.vector.tensor_scalar_add(
                            qkt[m][:, n * W:(n + 1) * W], ps, bqk[m])
                    nc.sync.dma_start(out=q2c[:, n * W:(n + 1) * W],
                                      in_=qkt[2][64:128, n * W:(n + 1) * W])

                for n in range(NQ):
                    emit_qk_block(n)

            with tc.tile_pool(name="ps_v", bufs=1, space="PSUM") as psV, \
                 tc.tile_pool(name="ps_e", bufs=1, space="PSUM") as psE, \
                 tc.tile_pool(name="ps_c", bufs=3, space="PSUM") as psC:

                def emit_v_block(n):
                    for s_ in range(4 * n, 4 * n + 4):
                        ps = psV.tile([P, W], f32, tag="projv",
                                      name=f"psv{s_}")
                        for k in range(KD):
                            nc.tensor.matmul(
                                ps[:, 0:256],
                                xt[k][:, s_ * P:(s_ + 1) * P],
                                wv[k],
                                start=(k == 0), stop=(k == KD - 1))
                        nc.vector.tensor_tensor(out=vsb[s_], in0=ps[:, 0:256],
                                                in1=bv, op=ALU.add)

                # head start: emit the first 3 attention groups' score
                # matmuls + exps BEFORE the V projection so the ACT stream
                # begins while the PE grinds through V. Their context
                # matmuls are deferred until V lands (depth-3 pipeline).
                HS = 3
                pending = []
                for gi, grp in enumerate(groups):
                    if gi == HS:
                        for n in range(NQ):
                            emit_v_block(n)
                    esb = emit_mm_s(gi, grp)
                    pending.append((grp, esb))
                    if gi >= HS:
                        emit_mm_c(*pending.pop(0))
                while pending:
                    emit_mm_c(*pending.pop(0))

                # ---- output projection: this core's 192 out-feature
                # columns of Wo, for every quarter. Quarters 0-2 have
                # their slabs in SBUF long ago; their projection keeps the
                # PE busy while AllGather #3 + slab #3 land.
                # PSUM slots: the two eps tags are retired after the last
                # exp, so their 4 banks are reused (2 halves each) plus
                # the ps_v bank: 5 rotating slots, no PE stalls.
                slots = []
                pv = psV.tile([P, W], f32, tag="projv", name="ppv")
                slots.append(pv)
                ea = psE.tile([P, 2 * W], f32, tag="ea", name="ppa")
                eb = psE.tile([P, 2 * W], f32, tag="eb", name="ppb")
                for t in (ea, eb):
                    slots.append(t[:, 0:W])
                    slots.append(t[:, W:2 * W])
                pv2 = psV.tile([P, W], f32, tag="projv", name="ppv2")
                slots.append(pv2)
                ea2 = psE.tile([P, 2 * W], f32, tag="ea", name="ppa2")
                slots.append(ea2[:, 0:W])
                slots.append(ea2[:, W:2 * W])

                si = 0
                for q in range(NQ):
                    for part in range(2):
                        pp = slots[si]
                        si += 1
                        lo = part * P
                        hi = min(192, lo + P)
                        rows = hi - lo
                        for k in range(KD):
                            nc.tensor.matmul(
                                pp[0:rows, :],
                                wo[k][:, lo:hi],
                                slab[q][k],
                                start=(k == 0), stop=(k == KD - 1))
                        yt = outp.tile([P, W], f32, tag="yt")
                        nc.vector.tensor_scalar_add(
                            yt[0:rows, :], pp[0:rows, :],
                            boA if part == 0 else boB)
                        nc.scalar.dma_start(
                            out=out[q * 192 + lo:q * 192 + hi, :],
                            in_=yt[0:rows, :])

    nc.compile()
    return nc


def _get_nc():
    if "nc" not in _CACHE:
        _install_profile_shim()
        _CACHE["nc"] = _build()
    return _CACHE["nc"]


def _make_in_maps(x, Wq, bq, Wk, bk, Wv, bv, Wo, bo):
    scale = np.float32(1.0 / np.sqrt(HD))
    f = np.float32
    x, Wq, bq, Wk, bk, Wv, bv, Wo, bo = [
        np.asarray(a, dtype=f) for a in (x, Wq, bq, Wk, bk, Wv, bv, Wo, bo)]

    in_maps = []
    for c in range(N_CORES):
        b = c // 4
        r = c % 4
        hs = r * HPC
        hh = [hs, hs + 1, hs + 2]

        def wc(Wm, h):
            return Wm[:, h * HD:(h + 1) * HD]

        def bc(bm, h):
            return bm[h * HD:(h + 1) * HD]

        xTb = np.ascontiguousarray(x[b].T)
        w_qk = np.concatenate(
            [wc(Wk, hh[0]), wc(Wk, hh[1]),
             wc(Wq, hh[0]) * scale, wc(Wq, hh[1]) * scale,
             wc(Wk, hh[2]), wc(Wq, hh[2]) * scale], axis=1)
        b_qk = np.concatenate(
            [bc(bk, hh[0]), bc(bk, hh[1]),
             bc(bq, hh[0]) * scale, bc(bq, hh[1]) * scale,
             bc(bk, hh[2]), bc(bq, hh[2]) * scale])[:, None]
        w_v = np.zeros((D, 256), dtype=f)
        b_v = np.zeros((1, 256), dtype=f)
        for i, h in enumerate(hh):
            w_v[:, i * 65:i * 65 + HD] = wc(Wv, h)
            b_v[0, i * 65:i * 65 + HD] = bc(bv, h)
            b_v[0, i * 65 + HD] = 1.0
        # gather indices: for quarter q, row i of the full-batch ctx slab
        # lives at cc_all[q*1536 + 768*b + i]
        i_feat = np.arange(D, dtype=np.uint32)
        g = np.concatenate(
            [q * 1536 + 768 * b + i_feat for q in range(NQ)])
        in_maps.append({
            "xT": np.ascontiguousarray(xTb),
            "w_qk": np.ascontiguousarray(w_qk),
            "b_qk": np.ascontiguousarray(b_qk),
            "w_v": w_v,
            "b_v": b_v,
            "w_o": np.ascontiguousarray(
                Wo[:, 192 * r:192 * (r + 1)]).astype(ml_dtypes.bfloat16),
            "b_o": np.ascontiguousarray(bo[192 * r:192 * (r + 1)][:, None]),
            "gidx": g.astype(np.uint32)[:, None],
            "zin": np.zeros((P, P), dtype=f),
        })
    return in_maps


def _assemble(results):
    y = np.empty((B, S, D), dtype=np.float32)
    for c in range(N_CORES):
        b = c // 4
        r = c % 4
        o = results[c]["out"]
        for nq in range(NQ):
            y[b, nq * W:(nq + 1) * W, 192 * r:192 * (r + 1)] = \
                o[nq * 192:(nq + 1) * 192, :].T
    return y


def kernel(x, Wq, bq, Wk, bk, Wv, bv, Wo, bo, _trace=False):
    from concourse.bass_utils import run_bass_kernel_spmd

    nc = _get_nc()
    in_maps = _make_in_maps(x, Wq, bq, Wk, bk, Wv, bv, Wo, bo)
    res = run_bass_kernel_spmd(nc, in_maps, list(range(N_CORES)),
                               trace=_trace)
    _CACHE["last_results"] = res
    return _assemble([{"out": res.results[c]["out"]} for c in range(N_CORES)])
